# revision 60
# baseline (speedup 1.0000x reference)
"""Trainium2 Bass kernel for PVT-style spatial-reduction attention.

Problem (per batch element b of 8, one NeuronCore each — pure data parallel):
  q  = x @ Wq + bq                                  [16384, 64]
  xs = conv8x8s8(x.reshape(128,128,64), Wsr) + bsr  [256, 64]
  xs = LayerNorm(xs) * gamma + beta
  k  = xs @ Wk + bk ; v = xs @ Wv + bv              [256, 64]
  A  = softmax(q @ k.T / 8) ; o = A @ v             [16384, 64]
  out = o @ Wp + bp

v4 design:
  - All weight-only folds on the HOST (gt2/wb/nxw/bvp/bsr/wsr2); x is
    host-pre-permuted into the on-chip xT2 layout in bf16:
      xT2[jp*64+c, (blk*8+pj)*128 + p] = x[blk*2048 + p*16 + pj*2 + jp, c]
    (no device transposes; input DMA halved to 2MB, 1 descriptor per
    partition on every DMA).
  - Everything on the PE is bf16.  Conv fuses tap pairs across the
    token-parity partition split via host-shifted wsr2.
  - LayerNorm stats via one bf16 ones-matmul over [xs | xs^2] (1/C and
    eps folded in); rstd = exp(-0.5*ln(var)).  A dummy Ln early in the
    preamble pre-loads the natural_log ACT table so only one table swap
    (to exp) ever sits on the critical path.
  - exp(S) split: ACT exact for ACT_COLS columns, DVE int16-Schraudolph
    for the rest: bf16_bits(e^(S/8)) ~= int16(S*23.083 + 16250.5) (the
    constant multiplicative ripple cancels in softmax).
  - Attention pipeline: y(ci-1) is emitted AFTER S(ci)/exp(ci) so the
    in-order PE queue never stalls on exp(ci) before starting S(ci+1).
    The vps (V-path) chain is emitted inside chunk 0 and its PSUM lives
    in the second (not-yet-used) bank of the first y tile — PSUM stays
    at exactly 8 banks.
  - Softmax normalization batched over 2 chunks; each chunk's 4 y-blocks
    sit in their own 512-f32 PSUM bank (a matmul output must never cross
    a bank boundary).  Output is bf16 (host casts back to f32).
"""

import os
import sys

import numpy as np

for _p in ("/root/.axon_site", "/root/.axon_site/_ro/trn_rl_repo",
           "/root/.axon_site/_ro/pypackages", "/opt/trn_rl_repo"):
    if os.path.isdir(_p) and _p not in sys.path:
        sys.path.append(_p)

import ml_dtypes  # noqa: E402

import concourse.bass as bass  # noqa: E402
import concourse.mybir as mybir  # noqa: E402
import concourse.tile as tile  # noqa: E402
from concourse import bacc  # noqa: E402
from concourse.bass_utils import run_bass_kernel_spmd  # noqa: E402
from concourse.masks import make_identity  # noqa: E402

F32 = mybir.dt.float32
F32R = mybir.dt.float32r
BF16 = mybir.dt.bfloat16
I16 = mybir.dt.int16
AF = mybir.ActivationFunctionType
ALU = mybir.AluOpType

N_CORES = 8
N = 16384          # tokens per core (H*W = 128*128)
C = 64             # channels
SR = 8
NKV = 256          # (128/8)^2
EPS = 1e-5
N_CHUNK = 512      # query tokens per attention chunk
N_CHUNKS = N // N_CHUNK  # 32
TOK_TILE = 128

# exp columns handled by ACT (exact); DVE Schraudolph does the rest.
ACT_COLS_EVEN = 800
ACT_COLS_ODD = 800
# Schraudolph: bf16_bits(exp(S/8)) ~= int16(S * (2^7/ln2)/8 + 127*2^7 - C)
EXP_A = 184.66496423378 / 8.0
EXP_B = 16250.5

NBF = 2 * C + 1 + C   # gt2 | wb | nxw columns in the bf16 const blob


def _patch_act_tables():
    """Bias the ACT-table-load insertion pass so Ln and Exp both resolve to
    the one act_info table that contains them both
    (natural_log_exp_and_others).  Table order/indices are untouched (walrus
    maps act_func_set_id by index); we only narrow the pass's coverage view,
    so the single load happens at the early dummy Ln and nothing reloads on
    the critical path."""
    from concourse import hw_specs
    orig = hw_specs.get_activation_tables

    def patched(arch):
        t = {name: set(fns) for name, fns in orig(arch).items()}
        for name, fns in t.items():
            if name != "natural_log_exp_and_others":
                fns.discard(AF.Exp)
                fns.discard(AF.Ln)
        return t

    bacc.get_activation_tables = patched


def build_graph():
    _patch_act_tables()
    nc = bacc.Bacc("TRN2", target_bir_lowering=False, debug=False,
                   num_devices=N_CORES)

    xt2_ext = nc.declare_dram_parameter("xt2", [128, N // 2], BF16,
                                        isOutput=False)
    wbf_ext = nc.declare_dram_parameter("wbf", [C, NBF], BF16, isOutput=False)
    wf32_ext = nc.declare_dram_parameter("wf32", [C, 2], F32, isOutput=False)
    wsr2_ext = nc.declare_dram_parameter("wsr2", [128, SR * SR * C], BF16,
                                         isOutput=False)
    out_ext = nc.declare_dram_parameter("out", [N, C], BF16, isOutput=True)

    with tile.TileContext(nc) as tc:
        with tc.tile_pool(name="const", bufs=1) as const_pool, \
             tc.tile_pool(name="persist", bufs=1) as persist_pool, \
             tc.tile_pool(name="work", bufs=2) as work_pool:

            # ---------- DMAs (all 1 descriptor/partition) ----------
            # The DMA engines fair-share bandwidth across in-flight queues,
            # so only the transfers conv needs first (q0, q1, wsr) are issued
            # now; q2/q3 are gated behind conv progress further below.
            xT2 = persist_pool.tile([128, N // 2], BF16, tag="xT2")
            QD = N // 8  # 2048 cols per DMA chunk
            nc.sync.dma_start(xT2[:, 0:2 * QD], xt2_ext[:, 0:2 * QD])
            wsr_sb = const_pool.tile([128, SR * SR, C], BF16, tag="wsr")
            nc.sync.dma_start(wsr_sb[:].rearrange("p t c -> p (t c)"),
                              wsr2_ext[:])
            wbf_sb = const_pool.tile([C, NBF], BF16, tag="wbf")
            nc.sync.dma_start(wbf_sb[:], wbf_ext[:])
            wf32_sb = const_pool.tile([C, 2], F32, tag="wf32")
            nc.sync.dma_start(wf32_sb[:], wf32_ext[:])

            gt2_sb = wbf_sb[:, 0:2 * C]
            wb_sb = wbf_sb[:, 2 * C:2 * C + 1]
            nxw_sb = wbf_sb[:, 2 * C + 1:NBF]
            bvp_sb = wf32_sb[:, 0:1]
            bsr_sb = wf32_sb[:, 1:2]

            identity = const_pool.tile([128, 128], F32)
            make_identity(nc, identity[:])

            eps_t = const_pool.tile([1, 1], F32, tag="eps")
            nc.gpsimd.memset(eps_t[:], EPS)
            # dummy Ln: pre-loads the natural_log table early; the LN-phase
            # rstd=exp(-0.5*ln(var)) then pays exactly one table swap (to
            # exp), after which attention needs no more loads.
            warm_t = const_pool.tile([1, 1], F32, tag="warm")
            nc.scalar.activation(warm_t[:], eps_t[:], AF.Ln)

            # stats stationary: 65 rows of 1/C (row 64 weights the eps row)
            ones65_st = const_pool.tile([C + 1, 1], F32, tag="ones65_st")
            nc.gpsimd.memset(ones65_st[:], 1.0 / C)
            ones65 = const_pool.tile([C + 1, 1], BF16, tag="ones65")
            nc.vector.tensor_copy(ones65[:], ones65_st[:])
            onesr_st = const_pool.tile([1, C], F32, tag="onesr_st")
            nc.gpsimd.memset(onesr_st[:], 1.0)
            onesr1 = const_pool.tile([1, C], BF16, tag="onesr1")
            nc.vector.tensor_copy(onesr1[:], onesr_st[:])

            xT2v = xT2[:].rearrange(
                "p (b jp dh i1 di jh) -> p b jp dh i1 di jh",
                b=8, jp=2, dh=4, i1=2, di=8, jh=8)

            ov = out_ext[:].rearrange("(b p ur j) f -> b p ur j f",
                                      b=8, p=TOK_TILE, ur=8, j=2)

            kq2 = persist_pool.tile([128, NKV], BF16, tag="kq2")
            vps = [persist_pool.tile([TOK_TILE, C + 1], BF16, tag=f"vps{h}",
                                     name=f"vps{h}")
                   for h in range(2)]

            with tc.tile_pool(name="pre_psum", bufs=2, space="PSUM") as pre_ps:
                # PE warm-up: junk transposes bridge the DMA wait so the HAM
                # clock gate is at 2.4 GHz when conv starts (and a keepalive
                # below holds it through the LN phase).
                # (real matmuls: transpose-mode doesn't register as PE
                # activity for the HAM clock gate)
                warm_ps = pre_ps.tile([128, 512], F32, tag="warm", bufs=1)
                for _ in range(6):
                    nc.tensor.matmul(warm_ps[:, 0:128], identity[:],
                                     identity[:], start=True, stop=True)

                xs_ps = pre_ps.tile([C, NKV], F32, tag="conv", bufs=1)

                def conv_half(bh):
                    # taps over blocks [4bh, 4bh+4) -> kv columns [128bh, +128)
                    for k, dj in enumerate(range(0, SR, 2)):
                        for di in range(SR):
                            tap = di * SR + dj
                            nc.tensor.matmul(
                                xs_ps[:, 128 * bh:128 * bh + 128],
                                wsr_sb[:, tap, :],
                                xT2v[:, 4 * bh:4 * bh + 4, :, dj // 2, :, di, :],
                                start=(k == 0 and di == 0),
                                stop=(k == SR // 2 - 1 and di == SR - 1))

                # gate q2/q3 xT2 DMAs behind the wsr DMA: dummy copies into
                # the DMA target regions create WAW ordering, so q0/q1/wsr
                # get full DMA bandwidth first (the engines fair-share);
                # q2/q3 then land while conv half 0 runs.
                nc.vector.tensor_copy(xT2[0:1, 2 * QD:2 * QD + 1],
                                      wsr_sb[0:1, 0, 0:1])
                nc.sync.dma_start(xT2[:, 2 * QD:3 * QD],
                                  xt2_ext[:, 2 * QD:3 * QD])
                nc.vector.tensor_copy(xT2[0:1, 3 * QD:3 * QD + 1],
                                      wsr_sb[0:1, 0, 1:2])
                nc.sync.dma_start(xT2[:, 3 * QD:4 * QD],
                                  xt2_ext[:, 3 * QD:4 * QD])

                conv_half(0)
                conv_half(1)

                # ---------- layernorm (stats via one bf16 ones-matmul) ------
                # xs2 = [xs | xs^2] bf16; row 64 = [0 | eps*C] so the 1/C
                # stationary yields m12 = [mu | E[x^2]+eps] in one matmul.
                xs2 = work_pool.tile([C + 1, 2 * NKV], BF16, tag="sq")
                nc.gpsimd.memset(xs2[C:C + 1, 0:NKV], 0.0)
                nc.gpsimd.memset(xs2[C:C + 1, NKV:2 * NKV], EPS * C)
                xs = xs2[0:C, 0:NKV]
                nc.vector.tensor_scalar_add(xs, xs_ps[:], bsr_sb)
                nc.scalar.activation(xs2[0:C, NKV:2 * NKV], xs_ps[:],
                                     AF.Square, bias=bsr_sb)
                m12_ps = pre_ps.tile([1, 2 * NKV], F32, tag="m12", bufs=1)
                nc.tensor.matmul(m12_ps[:], ones65[:], xs2[:], start=True,
                                 stop=True)
                mu2 = work_pool.tile([1, NKV], F32, tag="st_mu2")
                nc.scalar.activation(mu2[:], m12_ps[:, 0:NKV], AF.Square)
                var = work_pool.tile([1, NKV], F32, tag="st_var")
                nc.vector.scalar_tensor_tensor(
                    var[:], m12_ps[:, NKV:2 * NKV], 1.0, mu2[:],
                    op0=ALU.mult, op1=ALU.subtract)
                # rstd = exp(-0.5 ln(var)): DVE reciprocal is ~6ns/elem
                # (microcoded), so the ACT ln+exp pair wins even with the
                # one-time exp table load (which also comes OFF the exp0 path)
                lnv = work_pool.tile([1, NKV], F32, tag="st_lnv")
                nc.scalar.activation(lnv[:], var[:], AF.Ln)
                # ab = [rstd | -mu*rstd] bf16, broadcast by one K=1 matmul
                ab = work_pool.tile([1, 2 * NKV], BF16, tag="st_ab")
                nc.scalar.activation(ab[:, 0:NKV], lnv[:], AF.Exp, scale=-0.5)
                nc.vector.scalar_tensor_tensor(
                    ab[:, NKV:2 * NKV], m12_ps[:, 0:NKV], -1.0, ab[:, 0:NKV],
                    op0=ALU.mult, op1=ALU.mult)
                ab_ps = pre_ps.tile([C, 2 * NKV], F32, tag="ab", bufs=1)
                nc.tensor.matmul(ab_ps[:], onesr1[:], ab[:], start=True,
                                 stop=True)
                xsn = work_pool.tile([C, NKV], BF16, tag="xsn")
                nc.vector.tensor_mul(xsn[:], xs, ab_ps[:, 0:NKV])
                nc.vector.tensor_add(xsn[:], xsn[:], ab_ps[:, NKV:2 * NKV])

                # ---------- K (kq2) — gates the attention stream ----------
                kq_ps = pre_ps.tile([128, NKV], F32, tag="kq", bufs=1)
                nc.tensor.matmul(kq_ps[:], gt2_sb, xsn[:], start=True,
                                 stop=True)
                nc.scalar.copy(kq2[:], kq_ps[:])

            # ---------- attention ----------
            # chunk ci = xT2 free block [256ci, 256ci+256): even-parity tokens
            # on partitions 0:64, odd on 64:128.  E col layout per chunk:
            # par*512 + mh*256 + tok.  y(ci-1) is emitted after S(ci)/exp(ci)
            # so the in-order PE queue never waits on exp before S(ci+1).
            def emit_s_exp(ci, s_pool, e_pool):
                s_ps = s_pool.tile([TOK_TILE, 2 * N_CHUNK], F32, tag="S")
                xb = xT2[:, 256 * ci:256 * (ci + 1)]
                for par in range(2):   # bank `par`: tokens of parity par
                    o = C * par
                    for mh in range(2):
                        base = par * N_CHUNK + mh * 256
                        nc.tensor.matmul(s_ps[:, base:base + 256],
                                         kq2[o:o + C, mh * 128:(mh + 1) * 128],
                                         xb[o:o + C, :], start=True, stop=True)
                e_t = e_pool.tile([TOK_TILE, 2 * N_CHUNK], BF16, tag="E",
                                  bufs=6)
                a_cols = ACT_COLS_ODD if ci % 2 else ACT_COLS_EVEN
                nc.scalar.activation(e_t[:, 0:a_cols], s_ps[:, 0:a_cols],
                                     AF.Exp, scale=0.125)
                if a_cols < 2 * N_CHUNK:
                    nc.vector.tensor_scalar(
                        e_t[:, a_cols:2 * N_CHUNK].bitcast(I16),
                        s_ps[:, a_cols:2 * N_CHUNK],
                        EXP_A, EXP_B, op0=ALU.mult, op1=ALU.add)
                return e_t

            def emit_vps_chain(scratch):
                """V-path: PSUM carved from `scratch` (one free y bank)."""
                d_h = []
                for h in range(2):
                    bqk_ps = scratch[:, 384 + h:385 + h]
                    nc.tensor.matmul(bqk_ps,
                                     xsn[:, h * 128:(h + 1) * 128],
                                     wb_sb, start=True, stop=True)
                    dh = work_pool.tile([TOK_TILE, 1], F32, tag="dh")
                    nc.scalar.activation(dh[:], bqk_ps, AF.Exp, scale=0.125)
                    d_h.append(dh)
                vpT_ps = scratch[0:C, 128:384]
                nc.tensor.matmul(vpT_ps, nxw_sb, xsn[:], start=True, stop=True)
                vpT = work_pool.tile([C, NKV], F32, tag="vT_b")
                nc.vector.tensor_scalar_add(vpT[:], vpT_ps, bvp_sb)
                for h in range(2):
                    vpt_ps = scratch[:, 64 * h:64 * (h + 1)]
                    nc.tensor.transpose(vpt_ps, vpT[:, h * 128:(h + 1) * 128],
                                        identity[0:C, 0:C])
                    nc.vector.tensor_scalar_mul(vps[h][:, 0:C], vpt_ps,
                                                d_h[h][:])
                    nc.vector.tensor_copy(vps[h][:, C:C + 1], d_h[h][:])

            def emit_y(ci, y_ps):
                e_t = e_tiles[ci]
                for u in range(4):
                    ysl = y_ps[:, ci % 2, u * (C + 1):(u + 1) * (C + 1)]
                    b, j = u // 2, u % 2
                    col0 = 512 * j + 128 * b
                    nc.tensor.matmul(ysl, e_t[:, col0:col0 + 128],
                                     vps[0][:], start=True, stop=False)
                    nc.tensor.matmul(ysl, e_t[:, 256 + col0:256 + col0 + 128],
                                     vps[1][:], start=False, stop=True)

            def emit_norm(ci, y_ps):
                # normalize chunks (ci-1, ci) and DMA out
                yv = y_ps[:, :, 0:4 * (C + 1)].rearrange(
                    "p t (u q) -> p t u q", u=4)
                r_t = work_pool.tile([TOK_TILE, 2, 4, 1], F32, tag="r", bufs=2)
                nc.vector.reciprocal(r_t[:], yv[:, :, :, C:C + 1])
                y_t = work_pool.tile([TOK_TILE, 8, C], BF16, tag="y", bufs=2)
                nc.vector.tensor_mul(
                    y_t[:].rearrange("p (t u) f -> p t u f", t=2),
                    yv[:, :, :, 0:C],
                    r_t[:].broadcast_to([TOK_TILE, 2, 4, C]))
                g = ci // 2
                nc.sync.dma_start(
                    ov[g // 2, :, 4 * (g % 2):4 * (g % 2) + 4, :, :],
                    y_t[:].rearrange("p (s j) f -> p s j f", s=4, j=2))

            with tc.tile_pool(name="attn_psum_s", bufs=3, space="PSUM") as att_s, \
                 tc.tile_pool(name="attn_psum_y", bufs=1, space="PSUM") as att_y:
                e_tiles = {}
                y_tiles = {}

                def y_tile(k):
                    if k not in y_tiles:
                        y_tiles[k] = att_y.tile([TOK_TILE, 2, 512], F32,
                                                tag="Y", name=f"y{k}")
                    return y_tiles[k]

                for ci in range(N_CHUNKS):
                    e_tiles[ci] = emit_s_exp(ci, att_s, work_pool)
                    if ci == 0:
                        # vps chain uses the (empty) second bank of y tile 0
                        emit_vps_chain(y_tile(0)[:, 1, :])
                    if ci >= 1:
                        k = ci - 1
                        emit_y(k, y_tile(k // 2))
                        if k % 2 == 1:
                            emit_norm(k, y_tile(k // 2))
                    e_tiles.pop(ci - 2, None)
                emit_y(N_CHUNKS - 1, y_tile((N_CHUNKS - 1) // 2))
                emit_norm(N_CHUNKS - 1, y_tile((N_CHUNKS - 1) // 2))

    nc.finalize()
    return nc


_NC_CACHE = None


def _get_nc():
    global _NC_CACHE
    if _NC_CACHE is None:
        _NC_CACHE = build_graph()
    return _NC_CACHE


def _fold_weights(inputs):
    """Host-side weight folding (all pure functions of the weights)."""
    f32 = np.float32
    Wq = np.asarray(inputs["Wq"], f32)
    Wk = np.asarray(inputs["Wk"], f32)
    Wv = np.asarray(inputs["Wv"], f32)
    Wp = np.asarray(inputs["Wp"], f32)
    Wsr = np.asarray(inputs["Wsr"], f32)
    bq = np.asarray(inputs["bq"], f32)
    bv = np.asarray(inputs["bv"], f32)
    bsr = np.asarray(inputs["bsr"], f32)
    bp = np.asarray(inputs["bp"], f32)
    gamma = np.asarray(inputs["gamma"], f32)
    beta = np.asarray(inputs["beta"], f32)

    bf = ml_dtypes.bfloat16
    Wkg = gamma[:, None] * Wk
    Wvg = gamma[:, None] * Wv
    G = Wq @ Wkg.T                                   # [C, C]
    wbf = np.concatenate(
        [G.T, G.T, (Wkg @ bq)[:, None], Wvg @ Wp], axis=1)
    wbf = np.ascontiguousarray(wbf, dtype=bf)        # [C, 2C+1+C]
    bvp = (beta @ Wv + bv) @ Wp + bp
    wf32 = np.ascontiguousarray(
        np.stack([bvp, bsr], axis=1), dtype=f32)     # [C, 2]

    # wsr2: partition c holds tap t's weights for channel c; partition 64+c
    # holds tap t+1's weights (the odd-dj partner), so K=128 matmuls fuse
    # tap pairs across the token-parity partition split.
    wsr_f = Wsr.reshape(SR * SR, C, C)               # [tap, cin, cout]
    wsr2 = np.zeros((128, SR * SR, C), dtype=bf)
    wsr2[0:C] = np.swapaxes(wsr_f, 0, 1)             # [cin, tap, cout]
    wsr2[C:128, 0:SR * SR - 1] = np.swapaxes(wsr_f[1:], 0, 1)
    wsr2 = np.ascontiguousarray(wsr2.reshape(128, SR * SR * C))
    return dict(wbf=wbf, wf32=wf32, wsr2=wsr2)


def _make_in_maps(inputs):
    x = np.asarray(inputs["x"], dtype=np.float32)
    B = x.shape[0]
    assert x.shape == (B, N, C) and B == N_CORES, x.shape
    common = _fold_weights(inputs)
    bf = ml_dtypes.bfloat16
    x_bf = np.asarray(x, dtype=bf)
    in_maps = []
    for i in range(N_CORES):
        # xT2[jp*64+c, (blk*8+pj)*128+p] = x[blk*2048 + p*16 + pj*2 + jp, c]
        x3 = x_bf[i].reshape(8, 128, 8, 2, C)        # [blk, p, pj, jp, c]
        xt2 = np.ascontiguousarray(
            x3.transpose(3, 4, 0, 2, 1).reshape(128, N // 2))
        in_maps.append(dict(common, xt2=xt2))
    return in_maps


def run(inputs, trace=False):
    nc = _get_nc()
    in_maps = _make_in_maps(inputs)
    res = run_bass_kernel_spmd(nc, in_maps, list(range(N_CORES)), trace=trace)
    out = np.stack([np.asarray(res.results[i]["out"]) for i in range(N_CORES)])
    return out.astype(np.float32), res


def kernel(**inputs):
    out, _ = run(inputs, trace=False)
    return out


# revision 62
# speedup vs baseline: 1.0638x; 1.0638x over previous
"""Trainium2 Bass kernel for PVT-style spatial-reduction attention.

Problem (per batch element b of 8, one NeuronCore each — pure data parallel):
  q  = x @ Wq + bq                                  [16384, 64]
  xs = conv8x8s8(x.reshape(128,128,64), Wsr) + bsr  [256, 64]
  xs = LayerNorm(xs) * gamma + beta
  k  = xs @ Wk + bk ; v = xs @ Wv + bv              [256, 64]
  A  = softmax(q @ k.T / 8) ; o = A @ v             [16384, 64]
  out = o @ Wp + bp

v4 design:
  - All weight-only folds on the HOST (gt2/wb/nxw/bvp/bsr/wsr2); x is
    host-pre-permuted into the on-chip xT2 layout in bf16:
      xT2[jp*64+c, (blk*8+pj)*128 + p] = x[blk*2048 + p*16 + pj*2 + jp, c]
    (no device transposes; input DMA halved to 2MB, 1 descriptor per
    partition on every DMA).
  - Everything on the PE is bf16.  Conv fuses tap pairs across the
    token-parity partition split via host-shifted wsr2.
  - LayerNorm stats via one bf16 ones-matmul over [xs | xs^2] (1/C and
    eps folded in); rstd = exp(-0.5*ln(var)).  A dummy Ln early in the
    preamble pre-loads the natural_log ACT table so only one table swap
    (to exp) ever sits on the critical path.
  - exp(S) split: ACT exact for ACT_COLS columns, DVE int16-Schraudolph
    for the rest: bf16_bits(e^(S/8)) ~= int16(S*23.083 + 16250.5) (the
    constant multiplicative ripple cancels in softmax).
  - Attention pipeline: y(ci-1) is emitted AFTER S(ci)/exp(ci) so the
    in-order PE queue never stalls on exp(ci) before starting S(ci+1).
    The vps (V-path) chain is emitted inside chunk 0 and its PSUM lives
    in the second (not-yet-used) bank of the first y tile — PSUM stays
    at exactly 8 banks.
  - Softmax normalization batched over 2 chunks; each chunk's 4 y-blocks
    sit in their own 512-f32 PSUM bank (a matmul output must never cross
    a bank boundary).  Output is bf16 (host casts back to f32).
"""

import os
import sys

import numpy as np

for _p in ("/root/.axon_site", "/root/.axon_site/_ro/trn_rl_repo",
           "/root/.axon_site/_ro/pypackages", "/opt/trn_rl_repo"):
    if os.path.isdir(_p) and _p not in sys.path:
        sys.path.append(_p)

import ml_dtypes  # noqa: E402

import concourse.bass as bass  # noqa: E402
import concourse.mybir as mybir  # noqa: E402
import concourse.tile as tile  # noqa: E402
from concourse import bacc  # noqa: E402
from concourse.bass_utils import run_bass_kernel_spmd  # noqa: E402
from concourse.masks import make_identity  # noqa: E402

F32 = mybir.dt.float32
F32R = mybir.dt.float32r
BF16 = mybir.dt.bfloat16
I16 = mybir.dt.int16
AF = mybir.ActivationFunctionType
ALU = mybir.AluOpType

N_CORES = 8
N = 16384          # tokens per core (H*W = 128*128)
C = 64             # channels
SR = 8
NKV = 256          # (128/8)^2
EPS = 1e-5
N_CHUNK = 512      # query tokens per attention chunk
N_CHUNKS = N // N_CHUNK  # 32
TOK_TILE = 128

# exp columns handled by ACT (exact); DVE Schraudolph does the rest.
ACT_COLS_EVEN = 800
ACT_COLS_ODD = 800
# Schraudolph: bf16_bits(exp(S/8)) ~= int16(S * (2^7/ln2)/8 + 127*2^7 - C)
EXP_A = 184.66496423378 / 8.0
EXP_B = 16250.5

NBF = 2 * C + 1 + C   # gt2 | wb | nxw columns in the bf16 const blob


def _patch_act_tables():
    """Bias the ACT-table-load insertion pass so Ln and Exp both resolve to
    the one act_info table that contains them both
    (natural_log_exp_and_others).  Table order/indices are untouched (walrus
    maps act_func_set_id by index); we only narrow the pass's coverage view,
    so the single load happens at the early dummy Ln and nothing reloads on
    the critical path."""
    from concourse import hw_specs
    orig = hw_specs.get_activation_tables

    def patched(arch):
        t = {name: set(fns) for name, fns in orig(arch).items()}
        for name, fns in t.items():
            if name != "natural_log_exp_and_others":
                fns.discard(AF.Exp)
                fns.discard(AF.Ln)
        return t

    bacc.get_activation_tables = patched


def build_graph():
    _patch_act_tables()
    nc = bacc.Bacc("TRN2", target_bir_lowering=False, debug=False,
                   num_devices=N_CORES)

    xt2_ext = nc.declare_dram_parameter("xt2", [128, N // 2], BF16,
                                        isOutput=False)
    wbf_ext = nc.declare_dram_parameter("wbf", [C, NBF], BF16, isOutput=False)
    wf32_ext = nc.declare_dram_parameter("wf32", [C, 2], F32, isOutput=False)
    wsr2_ext = nc.declare_dram_parameter("wsr2", [128, SR * SR * C], BF16,
                                         isOutput=False)
    out_ext = nc.declare_dram_parameter("out", [N, C], BF16, isOutput=True)

    with tile.TileContext(nc) as tc:
        with tc.tile_pool(name="const", bufs=1) as const_pool, \
             tc.tile_pool(name="persist", bufs=1) as persist_pool, \
             tc.tile_pool(name="work", bufs=2) as work_pool:

            # ---------- DMAs (all 1 descriptor/partition) ----------
            # The DMA engines fair-share bandwidth across in-flight queues,
            # so only the transfers conv needs first (q0, q1, wsr) are issued
            # now; q2/q3 are gated behind conv progress further below.
            xT2 = persist_pool.tile([128, N // 2], BF16, tag="xT2")
            QD = N // 8  # 2048 cols per DMA chunk
            nc.sync.dma_start(xT2[:, 0:QD], xt2_ext[:, 0:QD])
            wsr_sb = const_pool.tile([128, SR * SR, C], BF16, tag="wsr")
            nc.sync.dma_start(wsr_sb[:].rearrange("p t c -> p (t c)"),
                              wsr2_ext[:])
            nc.sync.dma_start(xT2[:, QD:2 * QD], xt2_ext[:, QD:2 * QD])
            wbf_sb = const_pool.tile([C, NBF], BF16, tag="wbf")
            nc.sync.dma_start(wbf_sb[:], wbf_ext[:])
            wf32_sb = const_pool.tile([C, 2], F32, tag="wf32")
            nc.sync.dma_start(wf32_sb[:], wf32_ext[:])

            gt2_sb = wbf_sb[:, 0:2 * C]
            wb_sb = wbf_sb[:, 2 * C:2 * C + 1]
            nxw_sb = wbf_sb[:, 2 * C + 1:NBF]
            bvp_sb = wf32_sb[:, 0:1]
            bsr_sb = wf32_sb[:, 1:2]

            identity = const_pool.tile([128, 128], F32)
            make_identity(nc, identity[:])

            eps_t = const_pool.tile([1, 1], F32, tag="eps")
            nc.gpsimd.memset(eps_t[:], EPS)
            # dummy Ln: pre-loads the natural_log table early; the LN-phase
            # rstd=exp(-0.5*ln(var)) then pays exactly one table swap (to
            # exp), after which attention needs no more loads.
            warm_t = const_pool.tile([1, 1], F32, tag="warm")
            nc.scalar.activation(warm_t[:], eps_t[:], AF.Ln)

            # stats stationary: 65 rows of 1/C (row 64 weights the eps row)
            ones65_st = const_pool.tile([C + 1, 1], F32, tag="ones65_st")
            nc.gpsimd.memset(ones65_st[:], 1.0 / C)
            ones65 = const_pool.tile([C + 1, 1], BF16, tag="ones65")
            nc.vector.tensor_copy(ones65[:], ones65_st[:])
            onesr_st = const_pool.tile([1, C], F32, tag="onesr_st")
            nc.gpsimd.memset(onesr_st[:], 1.0)
            onesr1 = const_pool.tile([1, C], BF16, tag="onesr1")
            nc.vector.tensor_copy(onesr1[:], onesr_st[:])

            xT2v = xT2[:].rearrange(
                "p (b jp dh i1 di jh) -> p b jp dh i1 di jh",
                b=8, jp=2, dh=4, i1=2, di=8, jh=8)

            ov = out_ext[:].rearrange("(b p ur j) f -> b p ur j f",
                                      b=8, p=TOK_TILE, ur=8, j=2)

            kq2 = persist_pool.tile([128, NKV], BF16, tag="kq2")
            vps = [persist_pool.tile([TOK_TILE, C + 1], BF16, tag=f"vps{h}",
                                     name=f"vps{h}")
                   for h in range(2)]

            with tc.tile_pool(name="pre_psum", bufs=2, space="PSUM") as pre_ps:
                # PE warm-up: junk transposes bridge the DMA wait so the HAM
                # clock gate is at 2.4 GHz when conv starts (and a keepalive
                # below holds it through the LN phase).
                # (real matmuls: transpose-mode doesn't register as PE
                # activity for the HAM clock gate)
                warm_ps = pre_ps.tile([128, 512], F32, tag="warm", bufs=1)
                for _ in range(6):
                    nc.tensor.matmul(warm_ps[:, 0:128], identity[:],
                                     identity[:], start=True, stop=True)

                xs_ps = pre_ps.tile([C, NKV], F32, tag="conv", bufs=1)

                def conv_half(bh):
                    # taps over blocks [4bh, 4bh+4) -> kv columns [128bh, +128)
                    for k, dj in enumerate(range(0, SR, 2)):
                        for di in range(SR):
                            tap = di * SR + dj
                            nc.tensor.matmul(
                                xs_ps[:, 128 * bh:128 * bh + 128],
                                wsr_sb[:, tap, :],
                                xT2v[:, 4 * bh:4 * bh + 4, :, dj // 2, :, di, :],
                                start=(k == 0 and di == 0),
                                stop=(k == SR // 2 - 1 and di == SR - 1))

                # gate q2/q3 xT2 DMAs behind the wsr DMA: dummy copies into
                # the DMA target regions create WAW ordering, so q0/q1/wsr
                # get full DMA bandwidth first (the engines fair-share);
                # q2/q3 then land while conv half 0 runs.
                nc.vector.tensor_copy(xT2[0:1, 2 * QD:2 * QD + 1],
                                      wsr_sb[0:1, 0, 0:1])
                nc.sync.dma_start(xT2[:, 2 * QD:3 * QD],
                                  xt2_ext[:, 2 * QD:3 * QD])
                nc.vector.tensor_copy(xT2[0:1, 3 * QD:3 * QD + 1],
                                      wsr_sb[0:1, 0, 1:2])
                nc.sync.dma_start(xT2[:, 3 * QD:4 * QD],
                                  xt2_ext[:, 3 * QD:4 * QD])

                conv_half(0)
                conv_half(1)

                # ---------- layernorm (stats via one bf16 ones-matmul) ------
                # xs2 = [xs | xs^2] bf16; row 64 = [0 | eps*C] so the 1/C
                # stationary yields m12 = [mu | E[x^2]+eps] in one matmul.
                xs2 = work_pool.tile([C + 1, 2 * NKV], BF16, tag="sq")
                nc.gpsimd.memset(xs2[C:C + 1, 0:NKV], 0.0)
                nc.gpsimd.memset(xs2[C:C + 1, NKV:2 * NKV], EPS * C)
                xs = xs2[0:C, 0:NKV]
                nc.vector.tensor_scalar_add(xs, xs_ps[:], bsr_sb)
                nc.scalar.activation(xs2[0:C, NKV:2 * NKV], xs_ps[:],
                                     AF.Square, bias=bsr_sb)
                m12_ps = pre_ps.tile([1, 2 * NKV], F32, tag="m12", bufs=1)
                nc.tensor.matmul(m12_ps[:], ones65[:], xs2[:], start=True,
                                 stop=True)
                mu2 = work_pool.tile([1, NKV], F32, tag="st_mu2")
                nc.scalar.activation(mu2[:], m12_ps[:, 0:NKV], AF.Square)
                var = work_pool.tile([1, NKV], F32, tag="st_var")
                nc.vector.scalar_tensor_tensor(
                    var[:], m12_ps[:, NKV:2 * NKV], 1.0, mu2[:],
                    op0=ALU.mult, op1=ALU.subtract)
                # rstd = exp(-0.5 ln(var)): DVE reciprocal is ~6ns/elem
                # (microcoded), so the ACT ln+exp pair wins even with the
                # one-time exp table load (which also comes OFF the exp0 path)
                lnv = work_pool.tile([1, NKV], F32, tag="st_lnv")
                nc.scalar.activation(lnv[:], var[:], AF.Ln)
                # ab = [rstd | -mu*rstd] bf16, broadcast by one K=1 matmul
                ab = work_pool.tile([1, 2 * NKV], BF16, tag="st_ab")
                nc.scalar.activation(ab[:, 0:NKV], lnv[:], AF.Exp, scale=-0.5)
                nc.vector.scalar_tensor_tensor(
                    ab[:, NKV:2 * NKV], m12_ps[:, 0:NKV], -1.0, ab[:, 0:NKV],
                    op0=ALU.mult, op1=ALU.mult)
                ab_ps = pre_ps.tile([C, 2 * NKV], F32, tag="ab", bufs=1)
                nc.tensor.matmul(ab_ps[:], onesr1[:], ab[:], start=True,
                                 stop=True)
                xsn = work_pool.tile([C, NKV], BF16, tag="xsn")
                nc.vector.tensor_mul(xsn[:], xs, ab_ps[:, 0:NKV])
                nc.vector.tensor_add(xsn[:], xsn[:], ab_ps[:, NKV:2 * NKV])

                # ---------- K (kq2) — gates the attention stream ----------
                kq_ps = pre_ps.tile([128, NKV], F32, tag="kq", bufs=1)
                nc.tensor.matmul(kq_ps[:], gt2_sb, xsn[:], start=True,
                                 stop=True)
                nc.scalar.copy(kq2[:], kq_ps[:])

            # ---------- attention ----------
            # chunk ci = xT2 free block [256ci, 256ci+256): even-parity tokens
            # on partitions 0:64, odd on 64:128.  E col layout per chunk:
            # par*512 + mh*256 + tok.  y(ci-1) is emitted after S(ci)/exp(ci)
            # so the in-order PE queue never waits on exp before S(ci+1).
            def emit_s_exp(ci, s_pool, e_pool):
                s_ps = s_pool.tile([TOK_TILE, 2 * N_CHUNK], F32, tag="S")
                xb = xT2[:, 256 * ci:256 * (ci + 1)]
                for par in range(2):   # bank `par`: tokens of parity par
                    o = C * par
                    for mh in range(2):
                        base = par * N_CHUNK + mh * 256
                        nc.tensor.matmul(s_ps[:, base:base + 256],
                                         kq2[o:o + C, mh * 128:(mh + 1) * 128],
                                         xb[o:o + C, :], start=True, stop=True)
                e_t = e_pool.tile([TOK_TILE, 2 * N_CHUNK], BF16, tag="E",
                                  bufs=5)
                a_cols = ACT_COLS_ODD if ci % 2 else ACT_COLS_EVEN
                nc.scalar.activation(e_t[:, 0:a_cols], s_ps[:, 0:a_cols],
                                     AF.Exp, scale=0.125)
                if a_cols < 2 * N_CHUNK:
                    nc.vector.tensor_scalar(
                        e_t[:, a_cols:2 * N_CHUNK].bitcast(I16),
                        s_ps[:, a_cols:2 * N_CHUNK],
                        EXP_A, EXP_B, op0=ALU.mult, op1=ALU.add)
                return e_t

            def emit_vps_chain(scratch):
                """V-path: PSUM carved from `scratch` (one free y bank)."""
                d_h = []
                for h in range(2):
                    bqk_ps = scratch[:, 384 + h:385 + h]
                    nc.tensor.matmul(bqk_ps,
                                     xsn[:, h * 128:(h + 1) * 128],
                                     wb_sb, start=True, stop=True)
                    dh = work_pool.tile([TOK_TILE, 1], F32, tag="dh")
                    nc.scalar.activation(dh[:], bqk_ps, AF.Exp, scale=0.125)
                    d_h.append(dh)
                vpT_ps = scratch[0:C, 128:384]
                nc.tensor.matmul(vpT_ps, nxw_sb, xsn[:], start=True, stop=True)
                vpT = work_pool.tile([C, NKV], F32, tag="vT_b")
                nc.vector.tensor_scalar_add(vpT[:], vpT_ps, bvp_sb)
                for h in range(2):
                    vpt_ps = scratch[:, 64 * h:64 * (h + 1)]
                    nc.tensor.transpose(vpt_ps, vpT[:, h * 128:(h + 1) * 128],
                                        identity[0:C, 0:C])
                    nc.vector.tensor_scalar_mul(vps[h][:, 0:C], vpt_ps,
                                                d_h[h][:])
                    nc.vector.tensor_copy(vps[h][:, C:C + 1], d_h[h][:])

            def emit_y(ci, y_ps):
                e_t = e_tiles[ci]
                for u in range(4):
                    ysl = y_ps[:, ci % 2, u * (C + 1):(u + 1) * (C + 1)]
                    b, j = u // 2, u % 2
                    col0 = 512 * j + 128 * b
                    nc.tensor.matmul(ysl, e_t[:, col0:col0 + 128],
                                     vps[0][:], start=True, stop=False)
                    nc.tensor.matmul(ysl, e_t[:, 256 + col0:256 + col0 + 128],
                                     vps[1][:], start=False, stop=True)

            def emit_norm(ci, y_ps):
                # normalize chunks (ci-1, ci) and DMA out
                yv = y_ps[:, :, 0:4 * (C + 1)].rearrange(
                    "p t (u q) -> p t u q", u=4)
                r_t = work_pool.tile([TOK_TILE, 2, 4, 1], F32, tag="r", bufs=2)
                nc.vector.reciprocal(r_t[:], yv[:, :, :, C:C + 1])
                y_t = work_pool.tile([TOK_TILE, 8, C], BF16, tag="y", bufs=2)
                nc.vector.tensor_mul(
                    y_t[:].rearrange("p (t u) f -> p t u f", t=2),
                    yv[:, :, :, 0:C],
                    r_t[:].broadcast_to([TOK_TILE, 2, 4, C]))
                g = ci // 2
                nc.sync.dma_start(
                    ov[g // 2, :, 4 * (g % 2):4 * (g % 2) + 4, :, :],
                    y_t[:].rearrange("p (s j) f -> p s j f", s=4, j=2))

            with tc.tile_pool(name="attn_psum_s", bufs=3, space="PSUM") as att_s, \
                 tc.tile_pool(name="attn_psum_y", bufs=1, space="PSUM") as att_y:
                e_tiles = {}
                y_tiles = {}

                def y_tile(k):
                    if k not in y_tiles:
                        y_tiles[k] = att_y.tile([TOK_TILE, 2, 512], F32,
                                                tag="Y", name=f"y{k}")
                    return y_tiles[k]

                for ci in range(N_CHUNKS):
                    e_tiles[ci] = emit_s_exp(ci, att_s, work_pool)
                    if ci == 0:
                        # vps chain uses the (empty) second bank of y tile 0
                        emit_vps_chain(y_tile(0)[:, 1, :])
                    if ci >= 1:
                        k = ci - 1
                        emit_y(k, y_tile(k // 2))
                        if k % 2 == 1:
                            emit_norm(k, y_tile(k // 2))
                    e_tiles.pop(ci - 2, None)
                emit_y(N_CHUNKS - 1, y_tile((N_CHUNKS - 1) // 2))
                emit_norm(N_CHUNKS - 1, y_tile((N_CHUNKS - 1) // 2))

    nc.finalize()
    return nc


_NC_CACHE = None


def _get_nc():
    global _NC_CACHE
    if _NC_CACHE is None:
        _NC_CACHE = build_graph()
    return _NC_CACHE


def _fold_weights(inputs):
    """Host-side weight folding (all pure functions of the weights)."""
    f32 = np.float32
    Wq = np.asarray(inputs["Wq"], f32)
    Wk = np.asarray(inputs["Wk"], f32)
    Wv = np.asarray(inputs["Wv"], f32)
    Wp = np.asarray(inputs["Wp"], f32)
    Wsr = np.asarray(inputs["Wsr"], f32)
    bq = np.asarray(inputs["bq"], f32)
    bv = np.asarray(inputs["bv"], f32)
    bsr = np.asarray(inputs["bsr"], f32)
    bp = np.asarray(inputs["bp"], f32)
    gamma = np.asarray(inputs["gamma"], f32)
    beta = np.asarray(inputs["beta"], f32)

    bf = ml_dtypes.bfloat16
    Wkg = gamma[:, None] * Wk
    Wvg = gamma[:, None] * Wv
    G = Wq @ Wkg.T                                   # [C, C]
    wbf = np.concatenate(
        [G.T, G.T, (Wkg @ bq)[:, None], Wvg @ Wp], axis=1)
    wbf = np.ascontiguousarray(wbf, dtype=bf)        # [C, 2C+1+C]
    bvp = (beta @ Wv + bv) @ Wp + bp
    wf32 = np.ascontiguousarray(
        np.stack([bvp, bsr], axis=1), dtype=f32)     # [C, 2]

    # wsr2: partition c holds tap t's weights for channel c; partition 64+c
    # holds tap t+1's weights (the odd-dj partner), so K=128 matmuls fuse
    # tap pairs across the token-parity partition split.
    wsr_f = Wsr.reshape(SR * SR, C, C)               # [tap, cin, cout]
    wsr2 = np.zeros((128, SR * SR, C), dtype=bf)
    wsr2[0:C] = np.swapaxes(wsr_f, 0, 1)             # [cin, tap, cout]
    wsr2[C:128, 0:SR * SR - 1] = np.swapaxes(wsr_f[1:], 0, 1)
    wsr2 = np.ascontiguousarray(wsr2.reshape(128, SR * SR * C))
    return dict(wbf=wbf, wf32=wf32, wsr2=wsr2)


def _make_in_maps(inputs):
    x = np.asarray(inputs["x"], dtype=np.float32)
    B = x.shape[0]
    assert x.shape == (B, N, C) and B == N_CORES, x.shape
    common = _fold_weights(inputs)
    bf = ml_dtypes.bfloat16
    x_bf = np.asarray(x, dtype=bf)
    in_maps = []
    for i in range(N_CORES):
        # xT2[jp*64+c, (blk*8+pj)*128+p] = x[blk*2048 + p*16 + pj*2 + jp, c]
        x3 = x_bf[i].reshape(8, 128, 8, 2, C)        # [blk, p, pj, jp, c]
        xt2 = np.ascontiguousarray(
            x3.transpose(3, 4, 0, 2, 1).reshape(128, N // 2))
        in_maps.append(dict(common, xt2=xt2))
    return in_maps


def run(inputs, trace=False):
    nc = _get_nc()
    in_maps = _make_in_maps(inputs)
    res = run_bass_kernel_spmd(nc, in_maps, list(range(N_CORES)), trace=trace)
    out = np.stack([np.asarray(res.results[i]["out"]) for i in range(N_CORES)])
    return out.astype(np.float32), res


def kernel(**inputs):
    out, _ = run(inputs, trace=False)
    return out


# revision 63
# speedup vs baseline: 1.2294x; 1.1556x over previous
"""Trainium2 Bass kernel for PVT-style spatial-reduction attention.

Problem (per batch element b of 8, one NeuronCore each — pure data parallel):
  q  = x @ Wq + bq                                  [16384, 64]
  xs = conv8x8s8(x.reshape(128,128,64), Wsr) + bsr  [256, 64]
  xs = LayerNorm(xs) * gamma + beta
  k  = xs @ Wk + bk ; v = xs @ Wv + bv              [256, 64]
  A  = softmax(q @ k.T / 8) ; o = A @ v             [16384, 64]
  out = o @ Wp + bp

v4 design:
  - All weight-only folds on the HOST (gt2/wb/nxw/bvp/bsr/wsr2); x is
    host-pre-permuted into the on-chip xT2 layout in bf16:
      xT2[jp*64+c, (blk*8+pj)*128 + p] = x[blk*2048 + p*16 + pj*2 + jp, c]
    (no device transposes; input DMA halved to 2MB, 1 descriptor per
    partition on every DMA).
  - Everything on the PE is bf16.  Conv fuses tap pairs across the
    token-parity partition split via host-shifted wsr2.
  - LayerNorm stats via one bf16 ones-matmul over [xs | xs^2] (1/C and
    eps folded in); rstd = exp(-0.5*ln(var)).  A dummy Ln early in the
    preamble pre-loads the natural_log ACT table so only one table swap
    (to exp) ever sits on the critical path.
  - exp(S) split: ACT exact for ACT_COLS columns, DVE int16-Schraudolph
    for the rest: bf16_bits(e^(S/8)) ~= int16(S*23.083 + 16250.5) (the
    constant multiplicative ripple cancels in softmax).
  - Attention pipeline: y(ci-1) is emitted AFTER S(ci)/exp(ci) so the
    in-order PE queue never stalls on exp(ci) before starting S(ci+1).
    The vps (V-path) chain is emitted inside chunk 0 and its PSUM lives
    in the second (not-yet-used) bank of the first y tile — PSUM stays
    at exactly 8 banks.
  - Softmax normalization batched over 2 chunks; each chunk's 4 y-blocks
    sit in their own 512-f32 PSUM bank (a matmul output must never cross
    a bank boundary).  Output is bf16 (host casts back to f32).
"""

import os
import sys

import numpy as np

for _p in ("/root/.axon_site", "/root/.axon_site/_ro/trn_rl_repo",
           "/root/.axon_site/_ro/pypackages", "/opt/trn_rl_repo"):
    if os.path.isdir(_p) and _p not in sys.path:
        sys.path.append(_p)

import ml_dtypes  # noqa: E402

import concourse.bass as bass  # noqa: E402
import concourse.mybir as mybir  # noqa: E402
import concourse.tile as tile  # noqa: E402
from concourse import bacc  # noqa: E402
from concourse.bass_utils import run_bass_kernel_spmd  # noqa: E402
from concourse.masks import make_identity  # noqa: E402

F32 = mybir.dt.float32
F32R = mybir.dt.float32r
BF16 = mybir.dt.bfloat16
I16 = mybir.dt.int16
AF = mybir.ActivationFunctionType
ALU = mybir.AluOpType

N_CORES = 8
N = 16384          # tokens per core (H*W = 128*128)
C = 64             # channels
SR = 8
NKV = 256          # (128/8)^2
EPS = 1e-5
N_CHUNK = 512      # query tokens per attention chunk
N_CHUNKS = N // N_CHUNK  # 32
TOK_TILE = 128

# exp columns handled by ACT (exact); DVE Schraudolph does the rest.
ACT_COLS_EVEN = 800
ACT_COLS_ODD = 800
# Schraudolph: bf16_bits(exp(S/8)) ~= int16(S * (2^7/ln2)/8 + 127*2^7 - C)
EXP_A = 184.66496423378 / 8.0
EXP_B = 16250.5

NBF = 2 * C + 1 + C   # gt2 | wb | nxw columns in the bf16 const blob


def _patch_act_tables():
    """Bias the ACT-table-load insertion pass so Ln and Exp both resolve to
    the one act_info table that contains them both
    (natural_log_exp_and_others).  Table order/indices are untouched (walrus
    maps act_func_set_id by index); we only narrow the pass's coverage view,
    so the single load happens at the early dummy Ln and nothing reloads on
    the critical path."""
    from concourse import hw_specs
    orig = hw_specs.get_activation_tables

    def patched(arch):
        t = {name: set(fns) for name, fns in orig(arch).items()}
        for name, fns in t.items():
            if name != "natural_log_exp_and_others":
                fns.discard(AF.Exp)
                fns.discard(AF.Ln)
        return t

    bacc.get_activation_tables = patched


def build_graph():
    _patch_act_tables()
    nc = bacc.Bacc("TRN2", target_bir_lowering=False, debug=False,
                   num_devices=N_CORES)

    xt2_ext = nc.declare_dram_parameter("xt2", [128, N // 2], BF16,
                                        isOutput=False)
    wbf_ext = nc.declare_dram_parameter("wbf", [C, NBF], BF16, isOutput=False)
    wf32_ext = nc.declare_dram_parameter("wf32", [C, 2], F32, isOutput=False)
    wsr2_ext = nc.declare_dram_parameter("wsr2", [128, SR * SR * C], BF16,
                                         isOutput=False)
    out_ext = nc.declare_dram_parameter("out", [N, C], BF16, isOutput=True)

    with tile.TileContext(nc) as tc:
        with tc.tile_pool(name="const", bufs=1) as const_pool, \
             tc.tile_pool(name="persist", bufs=1) as persist_pool, \
             tc.tile_pool(name="work", bufs=2) as work_pool:

            # ---------- DMAs (all 1 descriptor/partition) ----------
            # The DMA engines fair-share bandwidth across in-flight queues,
            # so only the transfers conv needs first (q0, q1, wsr) are issued
            # now; q2/q3 are gated behind conv progress further below.
            xT2 = persist_pool.tile([128, N // 2], BF16, tag="xT2")
            QD = N // 8  # 2048 cols per DMA chunk
            nc.sync.dma_start(xT2[:, 0:QD], xt2_ext[:, 0:QD])
            wsr_sb = const_pool.tile([128, SR * SR, C], BF16, tag="wsr")
            nc.sync.dma_start(wsr_sb[:].rearrange("p t c -> p (t c)"),
                              wsr2_ext[:])
            nc.sync.dma_start(xT2[:, QD:2 * QD], xt2_ext[:, QD:2 * QD])
            wbf_sb = const_pool.tile([C, NBF], BF16, tag="wbf")
            nc.sync.dma_start(wbf_sb[:], wbf_ext[:])
            wf32_sb = const_pool.tile([C, 2], F32, tag="wf32")
            nc.sync.dma_start(wf32_sb[:], wf32_ext[:])

            gt2_sb = wbf_sb[:, 0:2 * C]
            wb_sb = wbf_sb[:, 2 * C:2 * C + 1]
            nxw_sb = wbf_sb[:, 2 * C + 1:NBF]
            bvp_sb = wf32_sb[:, 0:1]
            bsr_sb = wf32_sb[:, 1:2]

            identity = const_pool.tile([128, 128], F32)
            make_identity(nc, identity[:])

            eps_t = const_pool.tile([1, 1], F32, tag="eps")
            nc.gpsimd.memset(eps_t[:], EPS)
            # dummy Ln: pre-loads the natural_log table early; the LN-phase
            # rstd=exp(-0.5*ln(var)) then pays exactly one table swap (to
            # exp), after which attention needs no more loads.
            warm_t = const_pool.tile([1, 1], F32, tag="warm")
            nc.scalar.activation(warm_t[:], eps_t[:], AF.Ln)

            # stats stationary: 65 rows of 1/C (row 64 weights the eps row)
            ones65_st = const_pool.tile([C + 1, 1], F32, tag="ones65_st")
            nc.gpsimd.memset(ones65_st[:], 1.0 / C)
            ones65 = const_pool.tile([C + 1, 1], BF16, tag="ones65")
            nc.vector.tensor_copy(ones65[:], ones65_st[:])
            onesr_st = const_pool.tile([1, C], F32, tag="onesr_st")
            nc.gpsimd.memset(onesr_st[:], 1.0)
            onesr1 = const_pool.tile([1, C], BF16, tag="onesr1")
            nc.vector.tensor_copy(onesr1[:], onesr_st[:])

            xT2v = xT2[:].rearrange(
                "p (b jp dh i1 di jh) -> p b jp dh i1 di jh",
                b=8, jp=2, dh=4, i1=2, di=8, jh=8)

            ov = out_ext[:].rearrange("(b p ur j) f -> b p ur j f",
                                      b=8, p=TOK_TILE, ur=8, j=2)

            kq2 = persist_pool.tile([128, NKV], BF16, tag="kq2")
            vps = [persist_pool.tile([TOK_TILE, C + 1], BF16, tag=f"vps{h}",
                                     name=f"vps{h}")
                   for h in range(2)]

            with tc.tile_pool(name="pre_psum", bufs=2, space="PSUM") as pre_ps:
                # PE warm-up: junk transposes bridge the DMA wait so the HAM
                # clock gate is at 2.4 GHz when conv starts (and a keepalive
                # below holds it through the LN phase).
                # (real matmuls: transpose-mode doesn't register as PE
                # activity for the HAM clock gate)
                warm_ps = pre_ps.tile([128, 512], F32, tag="warm", bufs=1)
                for _ in range(6):
                    nc.tensor.matmul(warm_ps[:, 0:128], identity[:],
                                     identity[:], start=True, stop=True)

                xs_ps = pre_ps.tile([C, NKV], F32, tag="conv", bufs=1)

                def conv_half(bh):
                    # taps over blocks [4bh, 4bh+4) -> kv columns [128bh, +128)
                    for k, dj in enumerate(range(0, SR, 2)):
                        for di in range(SR):
                            tap = di * SR + dj
                            nc.tensor.matmul(
                                xs_ps[:, 128 * bh:128 * bh + 128],
                                wsr_sb[:, tap, :],
                                xT2v[:, 4 * bh:4 * bh + 4, :, dj // 2, :, di, :],
                                start=(k == 0 and di == 0),
                                stop=(k == SR // 2 - 1 and di == SR - 1))

                # gate q2/q3 xT2 DMAs behind the wsr DMA: dummy copies into
                # the DMA target regions create WAW ordering, so q0/q1/wsr
                # get full DMA bandwidth first (the engines fair-share);
                # q2/q3 then land while conv half 0 runs.
                nc.vector.tensor_copy(xT2[0:1, 2 * QD:2 * QD + 1],
                                      wsr_sb[0:1, 0, 0:1])
                nc.sync.dma_start(xT2[:, 2 * QD:3 * QD],
                                  xt2_ext[:, 2 * QD:3 * QD])
                nc.vector.tensor_copy(xT2[0:1, 3 * QD:3 * QD + 1],
                                      wsr_sb[0:1, 0, 1:2])
                nc.sync.dma_start(xT2[:, 3 * QD:4 * QD],
                                  xt2_ext[:, 3 * QD:4 * QD])

                conv_half(0)
                conv_half(1)

                # ---------- layernorm (stats via one bf16 ones-matmul) ------
                # xs2 = [xs | xs^2] bf16; row 64 = [0 | eps*C] so the 1/C
                # stationary yields m12 = [mu | E[x^2]+eps] in one matmul.
                xs2 = work_pool.tile([C + 1, 2 * NKV], BF16, tag="sq")
                nc.gpsimd.memset(xs2[C:C + 1, 0:NKV], 0.0)
                nc.gpsimd.memset(xs2[C:C + 1, NKV:2 * NKV], EPS * C)
                xs = xs2[0:C, 0:NKV]
                nc.vector.tensor_scalar_add(xs, xs_ps[:], bsr_sb)
                nc.scalar.activation(xs2[0:C, NKV:2 * NKV], xs_ps[:],
                                     AF.Square, bias=bsr_sb)
                m12_ps = pre_ps.tile([1, 2 * NKV], F32, tag="m12", bufs=1)
                nc.tensor.matmul(m12_ps[:], ones65[:], xs2[:], start=True,
                                 stop=True)
                mu2 = work_pool.tile([1, NKV], F32, tag="st_mu2")
                nc.scalar.activation(mu2[:], m12_ps[:, 0:NKV], AF.Square)
                var = work_pool.tile([1, NKV], F32, tag="st_var")
                nc.vector.scalar_tensor_tensor(
                    var[:], m12_ps[:, NKV:2 * NKV], 1.0, mu2[:],
                    op0=ALU.mult, op1=ALU.subtract)
                # rstd = exp(-0.5 ln(var)): DVE reciprocal is ~6ns/elem
                # (microcoded), so the ACT ln+exp pair wins even with the
                # one-time exp table load (which also comes OFF the exp0 path)
                lnv = work_pool.tile([1, NKV], F32, tag="st_lnv")
                nc.scalar.activation(lnv[:], var[:], AF.Ln)
                # ab = [rstd | -mu*rstd] bf16, broadcast by one K=1 matmul
                ab = work_pool.tile([1, 2 * NKV], BF16, tag="st_ab")
                nc.scalar.activation(ab[:, 0:NKV], lnv[:], AF.Exp, scale=-0.5)
                nc.vector.scalar_tensor_tensor(
                    ab[:, NKV:2 * NKV], m12_ps[:, 0:NKV], -1.0, ab[:, 0:NKV],
                    op0=ALU.mult, op1=ALU.mult)
                ab_ps = pre_ps.tile([C, 2 * NKV], F32, tag="ab", bufs=1)
                nc.tensor.matmul(ab_ps[:], onesr1[:], ab[:], start=True,
                                 stop=True)
                xsn = work_pool.tile([C, NKV], BF16, tag="xsn")
                nc.vector.tensor_mul(xsn[:], xs, ab_ps[:, 0:NKV])
                nc.vector.tensor_add(xsn[:], xsn[:], ab_ps[:, NKV:2 * NKV])

                # ---------- K (kq2) — gates the attention stream ----------
                kq_ps = pre_ps.tile([128, NKV], F32, tag="kq", bufs=1)
                nc.tensor.matmul(kq_ps[:], gt2_sb, xsn[:], start=True,
                                 stop=True)
                nc.scalar.copy(kq2[:], kq_ps[:])

            # ---------- attention ----------
            # chunk ci = xT2 free block [256ci, 256ci+256): even-parity tokens
            # on partitions 0:64, odd on 64:128.  E col layout per chunk:
            # par*512 + mh*256 + tok.  y(ci-1) is emitted after S(ci)/exp(ci)
            # so the in-order PE queue never waits on exp before S(ci+1).
            def emit_s_exp(ci, s_pool, e_pool):
                s_ps = s_pool.tile([TOK_TILE, 2 * N_CHUNK], F32, tag="S")
                if ci % 4 == 0:
                    # periodic full-array junk matmul (N=512, overwritten by
                    # the S par=0 matmuls via start=True): a chunky activity
                    # burst for the HAM clock gate, which otherwise flips the
                    # PE to 2.4 GHz late (or not at all) on the borderline
                    # K=64/N=65 attention matmul mix.
                    nc.tensor.matmul(s_ps[:, 0:N_CHUNK], kq2[:, 0:128],
                                     xT2[:, 0:N_CHUNK], start=True, stop=True)
                xb = xT2[:, 256 * ci:256 * (ci + 1)]
                for par in range(2):   # bank `par`: tokens of parity par
                    o = C * par
                    for mh in range(2):
                        base = par * N_CHUNK + mh * 256
                        nc.tensor.matmul(s_ps[:, base:base + 256],
                                         kq2[o:o + C, mh * 128:(mh + 1) * 128],
                                         xb[o:o + C, :], start=True, stop=True)
                e_t = e_pool.tile([TOK_TILE, 2 * N_CHUNK], BF16, tag="E",
                                  bufs=5)
                a_cols = ACT_COLS_ODD if ci % 2 else ACT_COLS_EVEN
                nc.scalar.activation(e_t[:, 0:a_cols], s_ps[:, 0:a_cols],
                                     AF.Exp, scale=0.125)
                if a_cols < 2 * N_CHUNK:
                    nc.vector.tensor_scalar(
                        e_t[:, a_cols:2 * N_CHUNK].bitcast(I16),
                        s_ps[:, a_cols:2 * N_CHUNK],
                        EXP_A, EXP_B, op0=ALU.mult, op1=ALU.add)
                return e_t

            def emit_vps_chain(scratch):
                """V-path: PSUM carved from `scratch` (one free y bank)."""
                d_h = []
                for h in range(2):
                    bqk_ps = scratch[:, 384 + h:385 + h]
                    nc.tensor.matmul(bqk_ps,
                                     xsn[:, h * 128:(h + 1) * 128],
                                     wb_sb, start=True, stop=True)
                    dh = work_pool.tile([TOK_TILE, 1], F32, tag="dh")
                    nc.scalar.activation(dh[:], bqk_ps, AF.Exp, scale=0.125)
                    d_h.append(dh)
                vpT_ps = scratch[0:C, 128:384]
                nc.tensor.matmul(vpT_ps, nxw_sb, xsn[:], start=True, stop=True)
                vpT = work_pool.tile([C, NKV], F32, tag="vT_b")
                nc.vector.tensor_scalar_add(vpT[:], vpT_ps, bvp_sb)
                for h in range(2):
                    vpt_ps = scratch[:, 64 * h:64 * (h + 1)]
                    nc.tensor.transpose(vpt_ps, vpT[:, h * 128:(h + 1) * 128],
                                        identity[0:C, 0:C])
                    nc.vector.tensor_scalar_mul(vps[h][:, 0:C], vpt_ps,
                                                d_h[h][:])
                    nc.vector.tensor_copy(vps[h][:, C:C + 1], d_h[h][:])

            def emit_y(ci, y_ps):
                e_t = e_tiles[ci]
                for u in range(4):
                    ysl = y_ps[:, ci % 2, u * (C + 1):(u + 1) * (C + 1)]
                    b, j = u // 2, u % 2
                    col0 = 512 * j + 128 * b
                    nc.tensor.matmul(ysl, e_t[:, col0:col0 + 128],
                                     vps[0][:], start=True, stop=False)
                    nc.tensor.matmul(ysl, e_t[:, 256 + col0:256 + col0 + 128],
                                     vps[1][:], start=False, stop=True)

            def emit_norm(ci, y_ps):
                # normalize chunks (ci-1, ci) and DMA out
                yv = y_ps[:, :, 0:4 * (C + 1)].rearrange(
                    "p t (u q) -> p t u q", u=4)
                r_t = work_pool.tile([TOK_TILE, 2, 4, 1], F32, tag="r", bufs=2)
                nc.vector.reciprocal(r_t[:], yv[:, :, :, C:C + 1])
                y_t = work_pool.tile([TOK_TILE, 8, C], BF16, tag="y", bufs=2)
                nc.vector.tensor_mul(
                    y_t[:].rearrange("p (t u) f -> p t u f", t=2),
                    yv[:, :, :, 0:C],
                    r_t[:].broadcast_to([TOK_TILE, 2, 4, C]))
                g = ci // 2
                nc.sync.dma_start(
                    ov[g // 2, :, 4 * (g % 2):4 * (g % 2) + 4, :, :],
                    y_t[:].rearrange("p (s j) f -> p s j f", s=4, j=2))

            with tc.tile_pool(name="attn_psum_s", bufs=3, space="PSUM") as att_s, \
                 tc.tile_pool(name="attn_psum_y", bufs=1, space="PSUM") as att_y:
                e_tiles = {}
                y_tiles = {}

                def y_tile(k):
                    if k not in y_tiles:
                        y_tiles[k] = att_y.tile([TOK_TILE, 2, 512], F32,
                                                tag="Y", name=f"y{k}")
                    return y_tiles[k]

                for ci in range(N_CHUNKS):
                    e_tiles[ci] = emit_s_exp(ci, att_s, work_pool)
                    if ci == 0:
                        # vps chain uses the (empty) second bank of y tile 0
                        emit_vps_chain(y_tile(0)[:, 1, :])
                    if ci >= 1:
                        k = ci - 1
                        emit_y(k, y_tile(k // 2))
                        if k % 2 == 1:
                            emit_norm(k, y_tile(k // 2))
                    e_tiles.pop(ci - 2, None)
                emit_y(N_CHUNKS - 1, y_tile((N_CHUNKS - 1) // 2))
                emit_norm(N_CHUNKS - 1, y_tile((N_CHUNKS - 1) // 2))

    nc.finalize()
    return nc


_NC_CACHE = None


def _get_nc():
    global _NC_CACHE
    if _NC_CACHE is None:
        _NC_CACHE = build_graph()
    return _NC_CACHE


def _fold_weights(inputs):
    """Host-side weight folding (all pure functions of the weights)."""
    f32 = np.float32
    Wq = np.asarray(inputs["Wq"], f32)
    Wk = np.asarray(inputs["Wk"], f32)
    Wv = np.asarray(inputs["Wv"], f32)
    Wp = np.asarray(inputs["Wp"], f32)
    Wsr = np.asarray(inputs["Wsr"], f32)
    bq = np.asarray(inputs["bq"], f32)
    bv = np.asarray(inputs["bv"], f32)
    bsr = np.asarray(inputs["bsr"], f32)
    bp = np.asarray(inputs["bp"], f32)
    gamma = np.asarray(inputs["gamma"], f32)
    beta = np.asarray(inputs["beta"], f32)

    bf = ml_dtypes.bfloat16
    Wkg = gamma[:, None] * Wk
    Wvg = gamma[:, None] * Wv
    G = Wq @ Wkg.T                                   # [C, C]
    wbf = np.concatenate(
        [G.T, G.T, (Wkg @ bq)[:, None], Wvg @ Wp], axis=1)
    wbf = np.ascontiguousarray(wbf, dtype=bf)        # [C, 2C+1+C]
    bvp = (beta @ Wv + bv) @ Wp + bp
    wf32 = np.ascontiguousarray(
        np.stack([bvp, bsr], axis=1), dtype=f32)     # [C, 2]

    # wsr2: partition c holds tap t's weights for channel c; partition 64+c
    # holds tap t+1's weights (the odd-dj partner), so K=128 matmuls fuse
    # tap pairs across the token-parity partition split.
    wsr_f = Wsr.reshape(SR * SR, C, C)               # [tap, cin, cout]
    wsr2 = np.zeros((128, SR * SR, C), dtype=bf)
    wsr2[0:C] = np.swapaxes(wsr_f, 0, 1)             # [cin, tap, cout]
    wsr2[C:128, 0:SR * SR - 1] = np.swapaxes(wsr_f[1:], 0, 1)
    wsr2 = np.ascontiguousarray(wsr2.reshape(128, SR * SR * C))
    return dict(wbf=wbf, wf32=wf32, wsr2=wsr2)


def _make_in_maps(inputs):
    x = np.asarray(inputs["x"], dtype=np.float32)
    B = x.shape[0]
    assert x.shape == (B, N, C) and B == N_CORES, x.shape
    common = _fold_weights(inputs)
    bf = ml_dtypes.bfloat16
    x_bf = np.asarray(x, dtype=bf)
    in_maps = []
    for i in range(N_CORES):
        # xT2[jp*64+c, (blk*8+pj)*128+p] = x[blk*2048 + p*16 + pj*2 + jp, c]
        x3 = x_bf[i].reshape(8, 128, 8, 2, C)        # [blk, p, pj, jp, c]
        xt2 = np.ascontiguousarray(
            x3.transpose(3, 4, 0, 2, 1).reshape(128, N // 2))
        in_maps.append(dict(common, xt2=xt2))
    return in_maps


def run(inputs, trace=False):
    nc = _get_nc()
    in_maps = _make_in_maps(inputs)
    res = run_bass_kernel_spmd(nc, in_maps, list(range(N_CORES)), trace=trace)
    out = np.stack([np.asarray(res.results[i]["out"]) for i in range(N_CORES)])
    return out.astype(np.float32), res


def kernel(**inputs):
    out, _ = run(inputs, trace=False)
    return out
